# revision 1
# baseline (speedup 1.0000x reference)
"""TRN2 Bass kernel for nn_Encoder: 2-layer LSTM encoder, batch-sharded over 8 cores.

Per core (B=2048): 4 batch groups x 512 in group-pair row layout (row = 32g+16*layer+j),
layer 2 staggered one step behind layer 1. Each round: gate planes (F,I,O,G) built by
PSUM accumulation: f32r relu(h)-projection (start), 4 concurrent col-tiled bf16
x-projections (one per group, tile_position=(32tq,32g), biases folded via const-1 pad
columns of x with a bf16-residual row), f32r h-recurrence (stop). Merged sigmoid over
3 PSUM banks; paired DVE elementwise; 2-way column split for cross-round pipelining.
x is cast to bf16 (18->32 feat pad) on-chip; one contiguous 64KB xbar DMA-transpose
per (group, 4-round quad) provides the K-side x tiles.
"""
import sys
sys.path.insert(0, '/opt/trn_rl_repo')
import numpy as np

B, T, F, H = 2048, 100, 18, 16
G = 4
BG = B // G
NS = 2
CW = BG // NS
N_CORES = 8
PLANE_SLICE = [16, 0, 48, 32]

_cache = {}

def _pack_weights(W_ih1, W_hh1, b_ih1, b_hh1, W_ih2, W_hh2, b_ih2, b_hh2):
    wqB = np.zeros((4, 128, 32), np.float32)   # mm1 lhsT: k=32tq+kk (+const@18), m=16l+j
    wr2 = np.zeros((4, 128, 128), np.float32)  # mm2 lhsT: k=relu(h) rows, m=L2 rows
    wh2 = np.zeros((4, 128, 128), np.float32)  # mm3 lhsT
    for X in range(4):
        s0 = PLANE_SLICE[X]
        b1 = b_ih1[s0:s0 + 16] + b_hh1[s0:s0 + 16]
        b2 = b_ih2[s0:s0 + 16] + b_hh2[s0:s0 + 16]
        import ml_dtypes
        b1a = b1.astype(ml_dtypes.bfloat16).astype(np.float32)
        b2a = b2.astype(ml_dtypes.bfloat16).astype(np.float32)
        for tq in range(4):
            wqB[X, 32 * tq:32 * tq + F, 0:16] = W_ih1[s0:s0 + 16, :].T
            wqB[X, 32 * tq + F, 0:16] = b1a
            wqB[X, 32 * tq + F, 16:32] = b2a
            wqB[X, 32 * tq + F + 1, 0:16] = b1 - b1a
            wqB[X, 32 * tq + F + 1, 16:32] = b2 - b2a
        for g in range(G):
            wr2[X, 32 * g:32 * g + 16, 32 * g + 16:32 * g + 32] = W_ih2[s0:s0 + 16, :].T
            wh2[X, 32 * g:32 * g + 16, 32 * g:32 * g + 16] = W_hh1[s0:s0 + 16, :].T
            wh2[X, 32 * g + 16:32 * g + 32, 32 * g + 16:32 * g + 32] = W_hh2[s0:s0 + 16, :].T
    wqL = np.zeros((4, 128, 32), np.float32)   # bias-only lhsT for the final round
    for X in range(4):
        s0 = PLANE_SLICE[X]
        import ml_dtypes
        b1 = b_ih1[s0:s0 + 16] + b_hh1[s0:s0 + 16]
        b2 = b_ih2[s0:s0 + 16] + b_hh2[s0:s0 + 16]
        b1a = b1.astype(ml_dtypes.bfloat16).astype(np.float32)
        b2a = b2.astype(ml_dtypes.bfloat16).astype(np.float32)
        wqL[X, 32 * 3 + F, 0:16] = b1a
        wqL[X, 32 * 3 + F, 16:32] = b2a
        wqL[X, 32 * 3 + F + 1, 0:16] = b1 - b1a
        wqL[X, 32 * 3 + F + 1, 16:32] = b2 - b2a
    mk = np.ones((128, 1), np.float32)         # L1 mask: zero the L2 rows
    for g in range(G):
        mk[32 * g + 16:32 * g + 32] = 0.0
    return wqB, wr2, wh2, wqL, mk


def _build():
    import concourse.bacc as bacc
    import concourse.tile as tile
    from concourse import mybir
    from concourse.masks import make_identity
    f32, f32r, bf16 = mybir.dt.float32, mybir.dt.float32r, mybir.dt.bfloat16
    AF, ALU = mybir.ActivationFunctionType, mybir.AluOpType
    R = T
    nc = bacc.Bacc(None, target_bir_lowering=False)
    x_d = nc.dram_tensor("x", [B, T, F], f32, kind="ExternalInput")
    wq_d = nc.dram_tensor("wq", [4, 128, 32], f32, kind="ExternalInput")
    wr_d = nc.dram_tensor("wr", [4, 128, 128], f32, kind="ExternalInput")
    wh_d = nc.dram_tensor("wh", [4, 128, 128], f32, kind="ExternalInput")
    wl_d = nc.dram_tensor("wl", [4, 128, 32], f32, kind="ExternalInput")
    mk_d = nc.dram_tensor("mk", [128, 1], f32, kind="ExternalInput")
    y_d = nc.dram_tensor("y", [B, H], f32, kind="ExternalOutput")

    with tile.TileContext(nc) as tc:
        with tc.tile_pool(name="singles", bufs=1) as singles, \
             tc.tile_pool(name="xf", bufs=4) as xfp, \
             tc.tile_pool(name="gq", bufs=4) as gqp, \
             tc.tile_pool(name="ps", bufs=3, space="PSUM") as psp, \
             tc.tile_pool(name="pso", bufs=2, space="PSUM") as psop, \
             tc.tile_pool(name="dram", bufs=1, space="DRAM") as drp:

            # ---- weights / constants ----
            wq_f = singles.tile([128, 4, 32], f32)
            nc.sync.dma_start(out=wq_f, in_=wq_d.rearrange("x k m -> k x m"))
            wq = singles.tile([128, 4, 32], bf16)
            nc.vector.tensor_copy(out=wq, in_=wq_f)
            wr_f = singles.tile([128, 4, 128], f32)
            nc.sync.dma_start(out=wr_f, in_=wr_d.rearrange("x k m -> k x m"))
            wr = singles.tile([128, 4, 128], f32r)
            nc.vector.tensor_copy(out=wr, in_=wr_f)
            wh_f = singles.tile([128, 4, 128], f32)
            nc.sync.dma_start(out=wh_f, in_=wh_d.rearrange("x k m -> k x m"))
            wh = singles.tile([128, 4, 128], f32r)
            nc.vector.tensor_copy(out=wh, in_=wh_f)
            wl_f = singles.tile([128, 4, 32], f32)
            nc.sync.dma_start(out=wl_f, in_=wl_d.rearrange("x k m -> k x m"))
            wl = singles.tile([128, 4, 32], bf16)
            nc.vector.tensor_copy(out=wl, in_=wl_f)
            mk = singles.tile([128, 1], f32)
            nc.sync.dma_start(out=mk, in_=mk_d[:, :])
            ident = singles.tile([128, 128], f32)
            make_identity(nc, ident)

            # ---- states (per column-half) ----
            h_h, cg_h, r1_h, s_h, tp_h, tn_h = [], [], [], [], [], []
            for i in range(NS):
                h_i = singles.tile([128, CW], f32r, tag=f"h{i}")
                nc.vector.memset(h_i.bitcast(f32), 0.0)
                h_h.append(h_i)
                cg_i = singles.tile([128, 2, CW], f32, tag=f"cg{i}")  # C | Gs
                nc.vector.memset(cg_i, 0.0)
                cg_h.append(cg_i)
                r1_i = singles.tile([128, CW], f32r, tag=f"r1{i}")    # relu(h) (full)
                nc.vector.memset(r1_i.bitcast(f32), 0.0)
                r1_h.append(r1_i)
                s_i = singles.tile([128, 3, CW], f32, tag=f"s{i}")
                s_h.append(s_i)
                tp_i = singles.tile([128, 2, CW], f32, tag=f"tp{i}")
                tp_h.append(tp_i)
                tn_i = singles.tile([128, CW], f32, tag=f"tn{i}")
                tn_h.append(tn_i)
            yrf = singles.tile([128, BG], f32)

            # ---- phase 0: cast x -> bf16, pad col 18 = 1.0 (bias const) ----
            x_padB = drp.tile([B, T, 32], bf16)
            stg = []
            for i in range(3):
                stg_i = singles.tile([128, T, 32], bf16, tag=f"stg{i}")
                stg.append(stg_i)
                nc.vector.memset(stg_i, 0.0)
                nc.vector.memset(stg_i[:, :, F:F + 2], 1.0)
            for c in range(B // 128):
                xf = xfp.tile([128, T, F], f32)
                nc.sync.dma_start(out=xf, in_=x_d[128 * c:128 * (c + 1), :, :])
                st = stg[c % 3]
                nc.vector.tensor_copy(out=st[:, :, 0:F], in_=xf)
                nc.sync.dma_start(out=x_padB[128 * c:128 * (c + 1), :, :], in_=st)

            # ---- quad transposes: 4 per 4 rounds, contiguous 64KB each ----
            quads = {}

            def emit_quad(q):
                if q in quads or 4 * q >= min(R, T):
                    return
                tiles = []
                for g in range(G):
                    gq_t = gqp.tile([128, BG], bf16, tag=f"gq{g}")
                    nc.sync.dma_start_transpose(
                        out=gq_t,
                        in_=x_padB[BG * g:BG * (g + 1), 4 * q:4 * q + 4, :]
                        .rearrange("b t k -> b (t k)"))
                    tiles.append(gq_t)
                quads[q] = tiles

            emit_quad(0)
            emit_quad(1)
            emit_quad(2)

            n_l1 = min(R, T)
            for r in range(R + 1):
                l1 = r < n_l1
                q, tq = r // 4, r % 4
                if l1 and tq == 2:
                    emit_quad(q + 2)
                if l1:
                    gq_cur = quads[q]
                else:
                    ql = (n_l1 - 1) // 4
                    gq_cur, tq = quads[ql], 3  # bias rides the last quad's const row
                for hf in range(NS):
                    cs = slice(CW * hf, CW * (hf + 1))
                    h_i, cg_i, r1_i, s_i, tp_i, tn_i = (
                        h_h[hf], cg_h[hf], r1_h[hf], s_h[hf], tp_h[hf], tn_h[hf])
                    P = psp.tile([128, 4, CW], f32)
                    for X in (0, 1, 3, 2):  # F, I first (sig-FI), G (tanhG), O last
                        nc.tensor.matmul(P[:, X, :], wr[:, X, :], r1_i,
                                         start=True, stop=False, skip_group_check=True)
                        for g in range(G):
                            if l1:
                                lhs = wq[32 * tq:32 * tq + F + 2, X, :]
                                rhs = gq_cur[g][32 * tq:32 * tq + F + 2, cs]
                            else:  # bias-only: zero lhsT except const row, via last quad
                                lhs = wl[32 * tq:32 * tq + F + 2, X, :]
                                rhs = gq_cur[g][32 * tq:32 * tq + F + 2, cs]
                            nc.tensor.matmul(P[32 * g:32 * g + 32, X, :], lhs, rhs,
                                             start=False, stop=False, skip_group_check=True,
                                             tile_position=(32 * tq, 32 * g))
                        nc.tensor.matmul(P[:, X, :], wh[:, X, :], h_i,
                                         start=False, stop=True, skip_group_check=True)
                    nc.scalar.activation(out=s_i[:, 0:2, :], in_=P[:, 0:2, :], func=AF.Sigmoid)
                    nc.scalar.activation(out=cg_i[:, 1, :], in_=P[:, 3, :], func=AF.Tanh)
                    nc.vector.tensor_tensor(out=tp_i[:, 0, :], in0=s_i[:, 0, :],
                                            in1=cg_i[:, 0, :], op=ALU.mult)
                    nc.gpsimd.tensor_tensor(out=tp_i[:, 1, :], in0=s_i[:, 1, :],
                                            in1=cg_i[:, 1, :], op=ALU.mult)
                    nc.scalar.activation(out=s_i[:, 2, :], in_=P[:, 2, :], func=AF.Sigmoid)
                    nc.vector.tensor_tensor(out=cg_i[:, 0, :], in0=tp_i[:, 0, :],
                                            in1=tp_i[:, 1, :], op=ALU.add)
                    nc.scalar.activation(out=tn_i, in_=cg_i[:, 0, :], func=AF.Tanh)
                    nc.vector.tensor_tensor(out=h_i, in0=s_i[:, 2, :],
                                            in1=tn_i, op=ALU.mult)
                    nc.gpsimd.tensor_scalar_max(out=r1_i, in0=h_i, scalar1=0.0)
                if r == 0:
                    # L2 starts at r=1 from zero state: mask-wipe its rows
                    for hf in range(NS):
                        nc.vector.tensor_scalar(out=h_h[hf], in0=h_h[hf], scalar1=mk,
                                                scalar2=None, op0=ALU.mult)
                        nc.vector.tensor_scalar(out=cg_h[hf][:, 0, :], in0=cg_h[hf][:, 0, :],
                                                scalar1=mk, scalar2=None, op0=ALU.mult)

            # ---- output: y = relu(h2) rows {32g+16..}, transposed out ----
            for hf in range(NS):
                nc.vector.tensor_scalar_max(out=yrf[:, CW * hf:CW * (hf + 1)],
                                            in0=h_h[hf], scalar1=0.0)
            for j in range(4):
                po = psop.tile([128, 128], f32)
                nc.tensor.transpose(po, yrf[:, 128 * j:128 * (j + 1)], ident)
                ys = xfp.tile([128, 128], f32, tag="ys")
                nc.vector.tensor_copy(out=ys, in_=po)
                for g in range(G):
                    nc.sync.dma_start(out=y_d[BG * g + 128 * j:BG * g + 128 * (j + 1), :],
                                      in_=ys[:, 32 * g + 16:32 * g + 32])
    nc.finalize()
    return nc




def kernel(x, W_ih1, W_hh1, b_ih1, b_hh1, W_ih2, W_hh2, b_ih2, b_hh2):
    from concourse.bass_utils import run_bass_kernel_spmd
    if "nc" not in _cache:
        _cache["nc"] = _build()
    nc = _cache["nc"]
    wq, wr, wh, wl, mk = _pack_weights(
        np.asarray(W_ih1), np.asarray(W_hh1), np.asarray(b_ih1), np.asarray(b_hh1),
        np.asarray(W_ih2), np.asarray(W_hh2), np.asarray(b_ih2), np.asarray(b_hh2))
    x = np.ascontiguousarray(np.asarray(x), dtype=np.float32)
    in_maps = [dict(x=x[i * B:(i + 1) * B], wq=wq, wr=wr, wh=wh, wl=wl, mk=mk)
               for i in range(N_CORES)]
    res = run_bass_kernel_spmd(nc, in_maps, core_ids=list(range(N_CORES)))
    return np.concatenate([res.results[i]["y"] for i in range(N_CORES)], axis=0)

